# revision 43
# baseline (speedup 1.0000x reference)
"""Multi-head causal attention (B=2, S=2048, D=2048, H=16) on 8 trn2 cores.

Sharding: tensor-parallel over heads. Core c owns heads 2c, 2c+1 (256
features of q/k/v). Each core computes its heads' QKV projections (+RoPE),
causal attention, and a partial output through its slice of wo. The 8
partial outputs are summed on the host (the "all-reduce").

All matmuls run in bf16 (f32 PSUM accumulation) — 1 cycle/column on the
PE vs ~2 for f32r. Key layout choices per core:

  qT, kT: [hd=128 partitions, head, token] bf16, head dims permuted
          (evens then odds) via host-permuted wq/wk rows so RoPE pairs
          sit in partition halves.
  v:      computed DIRECTLY in natural [token 128, feature 256] layout
          (x chunk is the stationary operand, wv the moving one), in two
          PSUM passes of 2 token-subchunks over the resident x tiles —
          no transposes, and the second pass gives the PE dependency-free
          work right before each attention phase.
  scores: per (key-chunk, 1024-query attention tile) as two f32 [128,512]
          single-bank MMs; exp on ACT -> bf16 probs; diagonal 128x128
          block masked on DVE. A running bf16 elementwise sum of prob
          chunks (DVE) feeds ONE denominator matmul per (head, tile)
          instead of one per chunk.
  attn:   accumulate attnT [hd, q] over key chunks in f32 PSUM (2 banks
          of 512 queries each), LAG-deep scores->attn pipelining; evict
          unnormalized to bf16 right away (frees banks). Denominator +
          normalize for BOTH heads are deferred past the second head's
          loop, with reserved wo work covering the latency; 1/Z via
          reciprocal_approx_fast on [1,512] + gpsimd partition-broadcast.
  wo:     o_ps [token 128, 512 j] f32 accumulates the 2 head-slices,
          evicted bf16 (DVE/ACT rotation), DMA'd from two queues; jobs
          drain as PE filler through the NEXT attention phase.

RoPE per accumulator [128, 512] (top 64 rows = even dims xr, bottom =
odd dims xi): ACT evicts psum -> qb bf16, then on DVE (bf16 2x modes,
sin table stored [-s; s] so both tensor_tensor inputs share a base
partition):
  p2[0:64] = qb[64:128]*s ; p2[64:128] = qb[0:64]*(-s)
  p1 = qb * [c; c] ;  dst = p1 - p2 = [xr*c - xi*s ; xi*c + xr*s]
"""

import math

import numpy as np

B = 2
S = 2048
D = 2048
H = 16
HD = 128
NCORES = 8
FPC = D // NCORES          # 256 features (2 heads) per core
P = 128
ND = D // P                # 16 contraction chunks
TT = 512                   # qkv token tile (f32 psum width)
NTT = S // TT              # 4 qkv tiles per batch
AT = 1024                  # attention query tile (bf16 psum width)
NAT = S // AT              # 2 attention tiles per batch
NKT = S // P               # 16 key chunks per batch
SCALE = 1.0 / math.sqrt(HD)
LAG = 3                    # scores -> attn pipeline depth (chunks)

_CACHE = {}


def _build_nc():
    import concourse.bass as bass  # noqa: F401
    from concourse import bacc
    import concourse.mybir as mybir
    import concourse.tile as tile

    f32 = mybir.dt.float32
    bf16 = mybir.dt.bfloat16
    MUL = mybir.AluOpType.mult
    SUB = mybir.AluOpType.subtract
    ADD = mybir.AluOpType.add
    EXP = mybir.ActivationFunctionType.Exp

    nc = bacc.Bacc(None, target_bir_lowering=False)

    xT = nc.dram_tensor("xT", [D, B * S], bf16, kind="ExternalInput")
    wqT = nc.dram_tensor("wqT", [D, FPC], bf16, kind="ExternalInput")
    wkT = nc.dram_tensor("wkT", [D, FPC], bf16, kind="ExternalInput")
    wvT = nc.dram_tensor("wvT", [D, FPC], bf16, kind="ExternalInput")
    woT = nc.dram_tensor("woT", [FPC, D], bf16, kind="ExternalInput")
    cosS = nc.dram_tensor("cosS", [P, S], bf16, kind="ExternalInput")
    sinS = nc.dram_tensor("sinS", [P, S], bf16, kind="ExternalInput")  # [-s; s]
    masks = nc.dram_tensor("masks", [P, P], bf16, kind="ExternalInput")
    onesd = nc.dram_tensor("onesd", [P, 1], bf16, kind="ExternalInput")
    identd = nc.dram_tensor("identd", [P, P], bf16, kind="ExternalInput")
    outp = nc.dram_tensor("outp", [B * S, D], bf16, kind="ExternalOutput")

    with tile.TileContext(nc) as tc:
        with (
            tc.tile_pool(name="res", bufs=1) as res,
            tc.tile_pool(name="xp", bufs=17) as xp,
            tc.tile_pool(name="csp", bufs=4) as csp,
            tc.tile_pool(name="ropep", bufs=2) as ropep,
            tc.tile_pool(name="probsp", bufs=8) as probsp,
            tc.tile_pool(name="prsump", bufs=2) as prsump,
            tc.tile_pool(name="aTp", bufs=2) as aTp,
            tc.tile_pool(name="aUp", bufs=4) as aUp,
            tc.tile_pool(name="rbp", bufs=2) as rbp,
            tc.tile_pool(name="dsbp", bufs=2) as dsbp,
            tc.tile_pool(name="ostp", bufs=6) as ostp,
            tc.tile_pool(name="ps_qk", bufs=4, space="PSUM") as ps_qk,
            tc.tile_pool(name="ps_vo", bufs=2, space="PSUM") as ps_vo,
            tc.tile_pool(name="ps_a", bufs=2, space="PSUM") as ps_a,
        ):
            # resident tensors; per-chunk weight tiles so each matmul's
            # DMA dependency is one small transfer, not the whole weight.
            wq_sb = [res.tile([P, FPC], bf16, name=f"wq{d}") for d in range(ND)]
            wk_sb = [res.tile([P, FPC], bf16, name=f"wk{d}") for d in range(ND)]
            wv_sb = [res.tile([P, FPC], bf16, name=f"wv{d}") for d in range(ND)]
            wo_sb = res.tile([P, 2, D], bf16)
            mask_sb = res.tile([P, P], bf16)
            ones_sb = res.tile([P, 1], bf16)
            qT_sb = res.tile([P, 2, S], bf16)
            kT_sb = res.tile([P, 2, S], bf16)
            v_sb = res.tile([P, NKT, FPC], bf16)
            ident = res.tile([P, P], bf16)

            def load_w_chunk(d):
                sl = slice(d * P, (d + 1) * P)
                weng = nc.scalar if d % 2 == 0 else nc.sync
                weng.dma_start(out=wq_sb[d][:], in_=wqT[sl, :])
                weng.dma_start(out=wk_sb[d][:], in_=wkT[sl, :])
                weng.dma_start(out=wv_sb[d][:], in_=wvT[sl, :])

            for d in range(4):
                load_w_chunk(d)
            nc.scalar.dma_start(out=ones_sb[:], in_=onesd[:])
            nc.scalar.dma_start(out=ident[:], in_=identd[:])
            nc.scalar.dma_start(out=mask_sb[:], in_=masks[:])

            # Warm-up: keep the PE busy (and ramp its p-state) while the
            # first weight/x DMAs land. Junk matmuls on a memset scratch
            # into the transpose slots; results are never read.
            scratch = res.tile([P, TT], bf16)
            nc.gpsimd.memset(scratch[:], 0.0)
            for _ in range(16):
                wu = ps_a.tile([P, TT], f32, name="av")
                for r in range(2):
                    nc.tensor.matmul(
                        wu[:], scratch[:, 0:P], scratch[:],
                        start=(r == 0), stop=(r == 1))

            wo_jobs = []
            out_eng = [nc.sync, nc.scalar]
            out_rr = [0]

            def emit_wo_group(trow0, aT, ts, jc):
                trow = trow0 + ts * P
                o_ps = ps_vo.tile([P, TT], f32, name="vo")
                for h in range(2):
                    nc.tensor.matmul(
                        o_ps[:],
                        aT[:, h, ts * P:(ts + 1) * P],
                        wo_sb[:, h, jc * TT:(jc + 1) * TT],
                        start=(h == 0), stop=(h == 1),
                    )
                ost = ostp.tile([P, TT], bf16, name="ost")
                if out_rr[0] % 3 != 2:
                    nc.vector.tensor_copy(ost[:], o_ps[:])
                else:
                    nc.scalar.copy(ost[:], o_ps[:])
                eng = out_eng[out_rr[0] % 2]
                out_rr[0] += 1
                eng.dma_start(
                    out=outp[trow:trow + P, jc * TT:(jc + 1) * TT],
                    in_=ost[:],
                )

            def drain_wo(n):
                for _ in range(n):
                    if wo_jobs:
                        emit_wo_group(*wo_jobs.pop(0))

            def emit_qkv(b, tt):
                t0g = b * S
                tsl = slice(tt * TT, (tt + 1) * TT)
                gsl = slice(t0g + tt * TT, t0g + (tt + 1) * TT)

                qk_ps = [ps_qk.tile([P, TT], f32, name="qs")
                         for _ in range(4)]
                # v computed directly in natural [token, feature] layout:
                # x chunk is the stationary operand, wv the moving one.
                # Two passes of 2 token-subchunks each (PSUM budget).
                v_ps = [ps_vo.tile([P, FPC], f32, name="vo")
                        for _ in range(2)]

                xts = []
                for d in range(ND):
                    if b == 0 and tt == 0 and d >= 4:
                        load_w_chunk(d)
                    xt = xp.tile([P, TT], bf16, name="xt")
                    xts.append(xt)
                    xeng = nc.sync if d % 2 == 0 else nc.gpsimd
                    xeng.dma_start(out=xt[:], in_=xT[d * P:(d + 1) * P, gsl])
                    if b == 0 and tt == 0 and d == ND - 1:
                        for fc in range(2):
                            nc.gpsimd.dma_start(
                                out=wo_sb[:, fc, :],
                                in_=woT[fc * P:(fc + 1) * P, :])
                    for fc in range(2):
                        nc.tensor.matmul(
                            qk_ps[fc][:],
                            wq_sb[d][:, fc * P:(fc + 1) * P],
                            xt[:],
                            start=(d == 0), stop=(d == ND - 1),
                        )
                        nc.tensor.matmul(
                            qk_ps[2 + fc][:],
                            wk_sb[d][:, fc * P:(fc + 1) * P],
                            xt[:],
                            start=(d == 0), stop=(d == ND - 1),
                        )
                    for sub in range(2):
                        nc.tensor.matmul(
                            v_ps[sub][:],
                            xt[:, sub * P:(sub + 1) * P],
                            wv_sb[d][:],
                            start=(d == 0), stop=(d == ND - 1),
                        )
                for sub in range(2):
                    nc.scalar.copy(v_sb[:, tt * 4 + sub, :], v_ps[sub][:])
                # second pass: token-subchunks 2,3 (x tiles resident)
                v_ps2 = [ps_vo.tile([P, FPC], f32, name="vo")
                         for _ in range(2)]
                for d in range(ND):
                    for i, sub in enumerate((2, 3)):
                        nc.tensor.matmul(
                            v_ps2[i][:],
                            xts[d][:, sub * P:(sub + 1) * P],
                            wv_sb[d][:],
                            start=(d == 0), stop=(d == ND - 1),
                        )
                for i, sub in enumerate((2, 3)):
                    nc.scalar.copy(v_sb[:, tt * 4 + sub, :], v_ps2[i][:])

                cct = csp.tile([P, TT], bf16, name="cct")
                sst = csp.tile([P, TT], bf16, name="sst")
                nc.gpsimd.dma_start(out=cct[:], in_=cosS[:, tsl])
                nc.gpsimd.dma_start(out=sst[:], in_=sinS[:, tsl])

                # RoPE: ACT evicts the psum accumulator to bf16, then
                # the rotation runs on DVE in bf16 (2x/4x modes, short
                # queue at the qkv->attention boundary). Head 0 (fc=0)
                # first: the next attention phase's first scores matmuls
                # depend only on those.
                for i, dst in ((0, qT_sb), (2, kT_sb), (1, qT_sb), (3, kT_sb)):
                    fc = i % 2
                    qb = ropep.tile([P, TT], bf16, name="qb")
                    nc.scalar.copy(qb[:], qk_ps[i][:])
                    p2 = ropep.tile([P, TT], bf16, name="p2")
                    nc.vector.tensor_tensor(
                        out=p2[0:64, :], in0=qb[64:128, :], in1=sst[64:128, :],
                        op=MUL)
                    nc.vector.tensor_tensor(
                        out=p2[64:128, :], in0=qb[0:64, :], in1=sst[0:64, :],
                        op=MUL)
                    p1 = ropep.tile([P, TT], bf16, name="p1")
                    nc.vector.tensor_tensor(
                        out=p1[:], in0=qb[:], in1=cct[:], op=MUL)
                    nc.vector.tensor_tensor(
                        out=dst[:, fc, tsl], in0=p1[:], in1=p2[:], op=SUB)

            def emit_attention(b, qt):
                # qt indexes AT=1024-query tiles; key chunks 0..nkt2-1
                # Burst of old-dependency PE work first (wo jobs of the
                # previous tile) to cover the fresh RoPE / exp latency of
                # this tile's first scores.
                drain_wo(8)
                t0g = b * S
                nkt2 = (AT // P) * qt + (AT // P)   # 8*qt + 8
                niter = 2 * (nkt2 + LAG)
                # reserve some wo jobs for the denominator/normalize section
                spread = max(0, len(wo_jobs) - 6)
                wo_per_iter = (spread + niter - 1) // niter if spread else 0
                aT = aTp.tile([P, 2, AT], bf16, name="aT")
                qbase = qt * AT
                prsums = [None, None]
                aUs = [None, None]

                for h in range(2):
                    a_ps = [ps_a.tile([P, TT], f32, name="av") for _ in range(2)]
                    prsum = prsums[h] = prsump.tile([P, AT], bf16, name="prsum")
                    pend = []
                    for kt in range(nkt2 + LAG):
                        drain_wo(wo_per_iter)
                        if kt < nkt2:
                            o = kt - (AT // P) * qt
                            c0 = max(o, 0) * P
                            pr = probsp.tile([P, AT], bf16, name="pr")
                            kch = kT_sb[:, h, kt * P:(kt + 1) * P]
                            for half in range(2):
                                lo = half * TT
                                ch = max(c0 - lo, 0)
                                if ch >= TT:
                                    continue
                                s_ps = ps_qk.tile([P, TT], f32, name="qs")
                                nc.tensor.matmul(
                                    s_ps[:, ch:],
                                    kch,
                                    qT_sb[:, h, qbase + lo + ch:
                                          qbase + lo + TT],
                                    start=True, stop=True,
                                )
                                nc.scalar.activation(
                                    pr[:, lo + ch:lo + TT], s_ps[:, ch:],
                                    EXP, scale=SCALE)
                            if o >= 0:
                                nc.vector.tensor_tensor(
                                    out=pr[:, c0:c0 + P],
                                    in0=pr[:, c0:c0 + P],
                                    in1=mask_sb[:],
                                    op=MUL,
                                )
                            if kt == 0:
                                nc.vector.tensor_copy(prsum[:], pr[:])
                            else:
                                nc.vector.tensor_tensor(
                                    out=prsum[:, c0:], in0=prsum[:, c0:],
                                    in1=pr[:, c0:], op=ADD)
                            pend.append((kt, c0, pr))
                        if kt >= LAG:
                            pkt, pc0, ppr = pend.pop(0)
                            vch = v_sb[:, pkt, h * P:(h + 1) * P]
                            # half 0: queries [pc0, 512)
                            if pc0 < TT:
                                last0 = (pkt == min(nkt2, (AT // P) * qt + 4) - 1)
                                nc.tensor.matmul(
                                    a_ps[0][:, pc0:],
                                    vch,
                                    ppr[:, pc0:TT],
                                    start=(pkt == 0), stop=last0,
                                )
                            # half 1: queries [max(pc0,512), 1024)
                            h1c0 = max(pc0 - TT, 0)
                            nc.tensor.matmul(
                                a_ps[1][:, h1c0:],
                                vch,
                                ppr[:, TT + h1c0:AT],
                                start=(pkt == 0), stop=(pkt == nkt2 - 1),
                            )
                    # Evict the unnormalized attention accumulators right
                    # away (frees the a_ps banks and decouples the denom
                    # critical path from this head's loop end).
                    aU = aUs[h] = aUp.tile([P, AT], bf16, name="aU")
                    nc.scalar.copy(aU[:, 0:TT], a_ps[0][:])
                    nc.scalar.copy(aU[:, TT:AT], a_ps[1][:])

                # Denominators + normalize, emitted after BOTH heads'
                # loops: head 0's prsum chain is long done, and wo work
                # reserved from the previous tile covers head 1's.
                for h in range(2):
                    drain_wo(3)
                    rb = rbp.tile([P, AT], f32, name="rb")
                    for half in range(2):
                        d_ps = ps_qk.tile([1, TT], f32, name="qs")
                        nc.tensor.matmul(
                            d_ps[:], ones_sb[:],
                            prsums[h][:, half * TT:(half + 1) * TT],
                            start=True, stop=True)
                        d_sb = dsbp.tile([1, TT], f32, name="dsb")
                        nc.vector.reciprocal_approx_fast(
                            out=d_sb[:], in_=d_ps[:])
                        nc.gpsimd.partition_broadcast(
                            rb[:, half * TT:(half + 1) * TT], d_sb[:])
                    nc.vector.tensor_tensor(
                        out=aT[:, h, :], in0=aUs[h][:], in1=rb[:], op=MUL)

                for ts in range(AT // P):
                    for jc in range(D // TT):
                        wo_jobs.append((t0g + qbase, aT, ts, jc))

            # schedule: qkv tiles stream; attention follows once its two
            # qkv tiles (and the previous tile's v transposes) are done.
            for b in range(B):
                emit_qkv(b, 0)
                emit_qkv(b, 1)
                emit_attention(b, 0)
                emit_qkv(b, 2)
                emit_qkv(b, 3)
                emit_attention(b, 1)
            drain_wo(len(wo_jobs))
    nc.compile()
    return nc


def _host_prep(x, wq, wk, wv, wo):
    import ml_dtypes

    bf16 = ml_dtypes.bfloat16
    x = np.asarray(x, dtype=np.float32)
    wq = np.asarray(wq, dtype=np.float32)
    wk = np.asarray(wk, dtype=np.float32)
    wv = np.asarray(wv, dtype=np.float32)
    wo = np.asarray(wo, dtype=np.float32)

    xT = np.ascontiguousarray(x.reshape(B * S, D).T).astype(bf16)  # [D, B*S]

    # permute q/k head dims: per head, even dims then odd dims
    perm = np.concatenate(
        [h * HD + np.concatenate([np.arange(0, HD, 2), np.arange(1, HD, 2)])
         for h in range(H)]
    )
    wq_p = wq[perm]
    wk_p = wk[perm]

    # rope tables; cos stacked twice, sin stacked [s; -s]
    inv_freq = 1.0 / (10000.0 ** (np.arange(0, HD, 2, dtype=np.float64) / HD))
    t = np.arange(S, dtype=np.float64)
    freqs = t[:, None] * inv_freq[None, :]            # [S, 64]
    cosT = np.cos(freqs).T.astype(np.float32)         # [64, S]
    sinT = np.sin(freqs).T.astype(np.float32)
    cosS = np.ascontiguousarray(np.vstack([cosT, cosT])).astype(bf16)
    sinS = np.ascontiguousarray(np.vstack([-sinT, sinT])).astype(bf16)

    # triangular causal mask for the diagonal 128x128 block
    pidx = np.arange(P)[:, None]
    qidx = np.arange(P)[None, :]
    m = np.ascontiguousarray((qidx >= pidx).astype(bf16))

    ones = np.ones((P, 1), dtype=bf16)

    in_maps = []
    for c in range(NCORES):
        fs = slice(c * FPC, (c + 1) * FPC)
        in_maps.append({
            "xT": xT,
            "wqT": np.ascontiguousarray(wq_p[fs].T).astype(bf16),   # [D, 256]
            "wkT": np.ascontiguousarray(wk_p[fs].T).astype(bf16),
            "wvT": np.ascontiguousarray(wv[fs].T).astype(bf16),
            "woT": np.ascontiguousarray(wo[:, fs].T).astype(bf16),  # [256, D]
            "cosS": cosS,
            "sinS": sinS,
            "masks": m,
            "onesd": ones,
            "identd": np.eye(P, dtype=bf16),
        })
    return in_maps


def _run(inputs, trace=False):
    from concourse.bass_utils import run_bass_kernel_spmd

    if "nc" not in _CACHE:
        _CACHE["nc"] = _build_nc()
    nc = _CACHE["nc"]

    in_maps = _host_prep(
        inputs["x"], inputs["wq"], inputs["wk"], inputs["wv"], inputs["wo"]
    )
    res = run_bass_kernel_spmd(nc, in_maps, list(range(NCORES)), trace=trace)
    acc = None
    for c in range(NCORES):
        part = np.asarray(res.results[c]["outp"], dtype=np.float32)
        acc = part.copy() if acc is None else acc + part
    out = acc.reshape(B, S, D).astype(np.float32)
    return out, res


def kernel(**inputs) -> np.ndarray:
    out, _ = _run(inputs, trace=False)
    return out


# revision 45
# speedup vs baseline: 1.1736x; 1.1736x over previous
"""Multi-head causal attention (B=2, S=2048, D=2048, H=16) on 8 trn2 cores.

Sharding: tensor-parallel over heads. Core c owns heads 2c, 2c+1 (256
features of q/k/v). Each core computes its heads' QKV projections (+RoPE),
causal attention, and a partial output through its slice of wo. The 8
partial outputs are summed on the host (the "all-reduce").

All matmuls run in bf16 (f32 PSUM accumulation) — 1 cycle/column on the
PE vs ~2 for f32r. Key layout choices per core:

  qT, kT: [hd=128 partitions, head, token] bf16, head dims permuted
          (evens then odds) via host-permuted wq/wk rows so RoPE pairs
          sit in partition halves.
  v:      computed DIRECTLY in natural [token 128, feature 256] layout
          (x chunk is the stationary operand, wv the moving one), in two
          PSUM passes of 2 token-subchunks over the resident x tiles —
          no transposes, and the second pass gives the PE dependency-free
          work right before each attention phase.
  scores: per (key-chunk, 1024-query attention tile) as two f32 [128,512]
          single-bank MMs; exp on ACT -> bf16 probs; diagonal 128x128
          block masked on DVE. A running bf16 elementwise sum of prob
          chunks (DVE) feeds ONE denominator matmul per (head, tile)
          instead of one per chunk.
  attn:   accumulate attnT [hd, q] over key chunks in f32 PSUM (2 banks
          of 512 queries each), LAG-deep scores->attn pipelining; evict
          unnormalized to bf16 right away (frees banks). Denominator +
          normalize for BOTH heads are deferred past the second head's
          loop, with reserved wo work covering the latency; 1/Z via
          reciprocal_approx_fast on [1,512] + gpsimd partition-broadcast.
  wo:     o_ps [token 128, 512 j] f32 accumulates the 2 head-slices,
          evicted bf16 (DVE/ACT rotation), DMA'd from two queues; jobs
          drain as PE filler through the NEXT attention phase.

RoPE per accumulator [128, 512] (top 64 rows = even dims xr, bottom =
odd dims xi): ACT evicts psum -> qb bf16, then on DVE (bf16 2x modes,
sin table stored [-s; s] so both tensor_tensor inputs share a base
partition):
  p2[0:64] = qb[64:128]*s ; p2[64:128] = qb[0:64]*(-s)
  p1 = qb * [c; c] ;  dst = p1 - p2 = [xr*c - xi*s ; xi*c + xr*s]
"""

import math

import numpy as np

B = 2
S = 2048
D = 2048
H = 16
HD = 128
NCORES = 8
FPC = D // NCORES          # 256 features (2 heads) per core
P = 128
ND = D // P                # 16 contraction chunks
TT = 512                   # qkv token tile (f32 psum width)
NTT = S // TT              # 4 qkv tiles per batch
AT = 1024                  # attention query tile (bf16 psum width)
NAT = S // AT              # 2 attention tiles per batch
NKT = S // P               # 16 key chunks per batch
SCALE = 1.0 / math.sqrt(HD)
LAG = 4                    # scores -> attn pipeline depth (chunks)

_CACHE = {}


def _build_nc():
    import concourse.bass as bass  # noqa: F401
    from concourse import bacc
    import concourse.mybir as mybir
    import concourse.tile as tile

    f32 = mybir.dt.float32
    bf16 = mybir.dt.bfloat16
    MUL = mybir.AluOpType.mult
    SUB = mybir.AluOpType.subtract
    ADD = mybir.AluOpType.add
    EXP = mybir.ActivationFunctionType.Exp

    nc = bacc.Bacc(None, target_bir_lowering=False)

    xT = nc.dram_tensor("xT", [D, B * S], bf16, kind="ExternalInput")
    wqT = nc.dram_tensor("wqT", [D, FPC], bf16, kind="ExternalInput")
    wkT = nc.dram_tensor("wkT", [D, FPC], bf16, kind="ExternalInput")
    wvT = nc.dram_tensor("wvT", [D, FPC], bf16, kind="ExternalInput")
    woT = nc.dram_tensor("woT", [FPC, D], bf16, kind="ExternalInput")
    cosS = nc.dram_tensor("cosS", [P, S], bf16, kind="ExternalInput")
    sinS = nc.dram_tensor("sinS", [P, S], bf16, kind="ExternalInput")  # [-s; s]
    masks = nc.dram_tensor("masks", [P, P], bf16, kind="ExternalInput")
    onesd = nc.dram_tensor("onesd", [P, 1], bf16, kind="ExternalInput")
    identd = nc.dram_tensor("identd", [P, P], bf16, kind="ExternalInput")
    outp = nc.dram_tensor("outp", [B * S, D], bf16, kind="ExternalOutput")

    with tile.TileContext(nc) as tc:
        with (
            tc.tile_pool(name="res", bufs=1) as res,
            tc.tile_pool(name="xp", bufs=17) as xp,
            tc.tile_pool(name="csp", bufs=4) as csp,
            tc.tile_pool(name="ropep", bufs=3) as ropep,
            tc.tile_pool(name="probsp", bufs=8) as probsp,
            tc.tile_pool(name="prsump", bufs=2) as prsump,
            tc.tile_pool(name="aTp", bufs=2) as aTp,
            tc.tile_pool(name="aUp", bufs=4) as aUp,
            tc.tile_pool(name="rbp", bufs=2) as rbp,
            tc.tile_pool(name="dsbp", bufs=2) as dsbp,
            tc.tile_pool(name="ostp", bufs=6) as ostp,
            tc.tile_pool(name="ps_qk", bufs=4, space="PSUM") as ps_qk,
            tc.tile_pool(name="ps_vo", bufs=2, space="PSUM") as ps_vo,
            tc.tile_pool(name="ps_a", bufs=2, space="PSUM") as ps_a,
        ):
            # resident tensors; per-chunk weight tiles so each matmul's
            # DMA dependency is one small transfer, not the whole weight.
            wq_sb = [res.tile([P, FPC], bf16, name=f"wq{d}") for d in range(ND)]
            wk_sb = [res.tile([P, FPC], bf16, name=f"wk{d}") for d in range(ND)]
            wv_sb = [res.tile([P, FPC], bf16, name=f"wv{d}") for d in range(ND)]
            wo_sb = res.tile([P, 2, D], bf16)
            mask_sb = res.tile([P, P], bf16)
            ones_sb = res.tile([P, 1], bf16)
            qT_sb = res.tile([P, 2, S], bf16)
            kT_sb = res.tile([P, 2, S], bf16)
            v_sb = res.tile([P, NKT, FPC], bf16)
            ident = res.tile([P, P], bf16)

            def load_w_chunk(d):
                sl = slice(d * P, (d + 1) * P)
                weng = nc.scalar if d % 2 == 0 else nc.sync
                weng.dma_start(out=wq_sb[d][:], in_=wqT[sl, :])
                weng.dma_start(out=wk_sb[d][:], in_=wkT[sl, :])
                weng.dma_start(out=wv_sb[d][:], in_=wvT[sl, :])

            for d in range(4):
                load_w_chunk(d)
            nc.scalar.dma_start(out=ones_sb[:], in_=onesd[:])
            nc.scalar.dma_start(out=ident[:], in_=identd[:])
            nc.scalar.dma_start(out=mask_sb[:], in_=masks[:])

            # Warm-up: keep the PE busy (and ramp its p-state) while the
            # first weight/x DMAs land. Junk matmuls on a memset scratch
            # into the transpose slots; results are never read.
            scratch = res.tile([P, TT], bf16)
            nc.gpsimd.memset(scratch[:], 0.0)
            for _ in range(8):
                wu = ps_a.tile([P, TT], f32, name="av")
                for r in range(2):
                    nc.tensor.matmul(
                        wu[:], scratch[:, 0:P], scratch[:],
                        start=(r == 0), stop=(r == 1))

            wo_jobs = []
            out_eng = [nc.sync, nc.gpsimd, nc.scalar]
            out_rr = [0]

            def emit_wo_group(trow0, aT, ts, jc):
                trow = trow0 + ts * P
                o_ps = ps_vo.tile([P, TT], f32, name="vo")
                for h in range(2):
                    nc.tensor.matmul(
                        o_ps[:],
                        aT[:, h, ts * P:(ts + 1) * P],
                        wo_sb[:, h, jc * TT:(jc + 1) * TT],
                        start=(h == 0), stop=(h == 1),
                    )
                ost = ostp.tile([P, TT], bf16, name="ost")
                if out_rr[0] % 3 != 2:
                    nc.vector.tensor_copy(ost[:], o_ps[:])
                else:
                    nc.scalar.copy(ost[:], o_ps[:])
                eng = out_eng[out_rr[0] % 3]
                out_rr[0] += 1
                eng.dma_start(
                    out=outp[trow:trow + P, jc * TT:(jc + 1) * TT],
                    in_=ost[:],
                )

            def drain_wo(n):
                for _ in range(n):
                    if wo_jobs:
                        emit_wo_group(*wo_jobs.pop(0))

            def emit_qkv(b, tt):
                t0g = b * S
                tsl = slice(tt * TT, (tt + 1) * TT)
                gsl = slice(t0g + tt * TT, t0g + (tt + 1) * TT)

                qk_ps = [ps_qk.tile([P, TT], f32, name="qs")
                         for _ in range(4)]
                # v computed directly in natural [token, feature] layout:
                # x chunk is the stationary operand, wv the moving one.
                # Two passes of 2 token-subchunks each (PSUM budget).
                v_ps = [ps_vo.tile([P, FPC], f32, name="vo")
                        for _ in range(2)]

                xts = []
                for d in range(ND):
                    if b == 0 and tt == 0 and d >= 4:
                        load_w_chunk(d)
                    xt = xp.tile([P, TT], bf16, name="xt")
                    xts.append(xt)
                    xeng = nc.sync if d % 2 == 0 else nc.gpsimd
                    xeng.dma_start(out=xt[:], in_=xT[d * P:(d + 1) * P, gsl])
                    if b == 0 and tt == 0 and d == ND - 1:
                        for fc in range(2):
                            nc.gpsimd.dma_start(
                                out=wo_sb[:, fc, :],
                                in_=woT[fc * P:(fc + 1) * P, :])
                    for fc in range(2):
                        nc.tensor.matmul(
                            qk_ps[fc][:],
                            wq_sb[d][:, fc * P:(fc + 1) * P],
                            xt[:],
                            start=(d == 0), stop=(d == ND - 1),
                        )
                        nc.tensor.matmul(
                            qk_ps[2 + fc][:],
                            wk_sb[d][:, fc * P:(fc + 1) * P],
                            xt[:],
                            start=(d == 0), stop=(d == ND - 1),
                        )
                    for sub in range(2):
                        nc.tensor.matmul(
                            v_ps[sub][:],
                            xt[:, sub * P:(sub + 1) * P],
                            wv_sb[d][:],
                            start=(d == 0), stop=(d == ND - 1),
                        )
                for sub in range(2):
                    nc.scalar.copy(v_sb[:, tt * 4 + sub, :], v_ps[sub][:])
                # second pass: token-subchunks 2,3 (x tiles resident)
                v_ps2 = [ps_vo.tile([P, FPC], f32, name="vo")
                         for _ in range(2)]
                for d in range(ND):
                    for i, sub in enumerate((2, 3)):
                        nc.tensor.matmul(
                            v_ps2[i][:],
                            xts[d][:, sub * P:(sub + 1) * P],
                            wv_sb[d][:],
                            start=(d == 0), stop=(d == ND - 1),
                        )
                for i, sub in enumerate((2, 3)):
                    nc.scalar.copy(v_sb[:, tt * 4 + sub, :], v_ps2[i][:])

                cct = csp.tile([P, TT], bf16, name="cct")
                sst = csp.tile([P, TT], bf16, name="sst")
                nc.gpsimd.dma_start(out=cct[:], in_=cosS[:, tsl])
                nc.gpsimd.dma_start(out=sst[:], in_=sinS[:, tsl])

                # RoPE: ACT evicts the psum accumulator to bf16, then
                # the rotation runs on DVE in bf16 (2x/4x modes, short
                # queue at the qkv->attention boundary). Head 0 (fc=0)
                # first: the next attention phase's first scores matmuls
                # depend only on those.
                for i, dst in ((0, qT_sb), (2, kT_sb), (1, qT_sb), (3, kT_sb)):
                    fc = i % 2
                    qb = ropep.tile([P, TT], bf16, name="qb")
                    nc.scalar.copy(qb[:], qk_ps[i][:])
                    p2 = ropep.tile([P, TT], bf16, name="p2")
                    nc.vector.tensor_tensor(
                        out=p2[0:64, :], in0=qb[64:128, :], in1=sst[64:128, :],
                        op=MUL)
                    nc.vector.tensor_tensor(
                        out=p2[64:128, :], in0=qb[0:64, :], in1=sst[0:64, :],
                        op=MUL)
                    p1 = ropep.tile([P, TT], bf16, name="p1")
                    nc.vector.tensor_tensor(
                        out=p1[:], in0=qb[:], in1=cct[:], op=MUL)
                    nc.vector.tensor_tensor(
                        out=dst[:, fc, tsl], in0=p1[:], in1=p2[:], op=SUB)

            def emit_attention(b, qt):
                # qt indexes AT=1024-query tiles; key chunks 0..nkt2-1
                # Burst of old-dependency PE work first (wo jobs of the
                # previous tile) to cover the fresh RoPE / exp latency of
                # this tile's first scores.
                drain_wo(10)
                t0g = b * S
                nkt2 = (AT // P) * qt + (AT // P)   # 8*qt + 8
                niter = 2 * (nkt2 + LAG)
                # reserve some wo jobs for the denominator/normalize section
                spread = max(0, len(wo_jobs) - 8)
                wo_per_iter = (spread + niter - 1) // niter if spread else 0
                aT = aTp.tile([P, 2, AT], bf16, name="aT")
                qbase = qt * AT
                prsums = [None, None]
                aUs = [None, None]

                for h in range(2):
                    a_ps = [ps_a.tile([P, TT], f32, name="av") for _ in range(2)]
                    prsum = prsums[h] = prsump.tile([P, AT], bf16, name="prsum")
                    pend = []
                    for kt in range(nkt2 + LAG):
                        drain_wo(wo_per_iter)
                        if kt < nkt2:
                            o = kt - (AT // P) * qt
                            c0 = max(o, 0) * P
                            pr = probsp.tile([P, AT], bf16, name="pr")
                            kch = kT_sb[:, h, kt * P:(kt + 1) * P]
                            for half in range(2):
                                lo = half * TT
                                ch = max(c0 - lo, 0)
                                if ch >= TT:
                                    continue
                                s_ps = ps_qk.tile([P, TT], f32, name="qs")
                                nc.tensor.matmul(
                                    s_ps[:, ch:],
                                    kch,
                                    qT_sb[:, h, qbase + lo + ch:
                                          qbase + lo + TT],
                                    start=True, stop=True,
                                )
                                nc.scalar.activation(
                                    pr[:, lo + ch:lo + TT], s_ps[:, ch:],
                                    EXP, scale=SCALE)
                            if o >= 0:
                                nc.vector.tensor_tensor(
                                    out=pr[:, c0:c0 + P],
                                    in0=pr[:, c0:c0 + P],
                                    in1=mask_sb[:],
                                    op=MUL,
                                )
                            if kt == 0:
                                nc.vector.tensor_copy(prsum[:], pr[:])
                            else:
                                nc.vector.tensor_tensor(
                                    out=prsum[:, c0:], in0=prsum[:, c0:],
                                    in1=pr[:, c0:], op=ADD)
                            pend.append((kt, c0, pr))
                        if kt >= LAG:
                            pkt, pc0, ppr = pend.pop(0)
                            vch = v_sb[:, pkt, h * P:(h + 1) * P]
                            # half 0: queries [pc0, 512)
                            if pc0 < TT:
                                last0 = (pkt == min(nkt2, (AT // P) * qt + 4) - 1)
                                nc.tensor.matmul(
                                    a_ps[0][:, pc0:],
                                    vch,
                                    ppr[:, pc0:TT],
                                    start=(pkt == 0), stop=last0,
                                )
                            # half 1: queries [max(pc0,512), 1024)
                            h1c0 = max(pc0 - TT, 0)
                            nc.tensor.matmul(
                                a_ps[1][:, h1c0:],
                                vch,
                                ppr[:, TT + h1c0:AT],
                                start=(pkt == 0), stop=(pkt == nkt2 - 1),
                            )
                    # Evict the unnormalized attention accumulators right
                    # away (frees the a_ps banks and decouples the denom
                    # critical path from this head's loop end).
                    aU = aUs[h] = aUp.tile([P, AT], bf16, name="aU")
                    nc.scalar.copy(aU[:, 0:TT], a_ps[0][:])
                    nc.scalar.copy(aU[:, TT:AT], a_ps[1][:])

                # Denominators + normalize, emitted after BOTH heads'
                # loops: head 0's prsum chain is long done, and wo work
                # reserved from the previous tile covers head 1's.
                for h in range(2):
                    drain_wo(3)
                    rb = rbp.tile([P, AT], f32, name="rb")
                    for half in range(2):
                        d_ps = ps_qk.tile([1, TT], f32, name="qs")
                        nc.tensor.matmul(
                            d_ps[:], ones_sb[:],
                            prsums[h][:, half * TT:(half + 1) * TT],
                            start=True, stop=True)
                        d_sb = dsbp.tile([1, TT], f32, name="dsb")
                        nc.vector.reciprocal_approx_fast(
                            out=d_sb[:], in_=d_ps[:])
                        nc.gpsimd.partition_broadcast(
                            rb[:, half * TT:(half + 1) * TT], d_sb[:])
                    nc.vector.tensor_tensor(
                        out=aT[:, h, :], in0=aUs[h][:], in1=rb[:], op=MUL)

                for ts in range(AT // P):
                    for jc in range(D // TT):
                        wo_jobs.append((t0g + qbase, aT, ts, jc))

            # schedule: qkv tiles stream; attention follows once its two
            # qkv tiles (and the previous tile's v transposes) are done.
            for b in range(B):
                emit_qkv(b, 0)
                emit_qkv(b, 1)
                emit_attention(b, 0)
                emit_qkv(b, 2)
                emit_qkv(b, 3)
                emit_attention(b, 1)
            drain_wo(len(wo_jobs))
    nc.compile()
    return nc


def _host_prep(x, wq, wk, wv, wo):
    import ml_dtypes

    bf16 = ml_dtypes.bfloat16
    x = np.asarray(x, dtype=np.float32)
    wq = np.asarray(wq, dtype=np.float32)
    wk = np.asarray(wk, dtype=np.float32)
    wv = np.asarray(wv, dtype=np.float32)
    wo = np.asarray(wo, dtype=np.float32)

    xT = np.ascontiguousarray(x.reshape(B * S, D).T).astype(bf16)  # [D, B*S]

    # permute q/k head dims: per head, even dims then odd dims
    perm = np.concatenate(
        [h * HD + np.concatenate([np.arange(0, HD, 2), np.arange(1, HD, 2)])
         for h in range(H)]
    )
    wq_p = wq[perm]
    wk_p = wk[perm]

    # rope tables; cos stacked twice, sin stacked [s; -s]
    inv_freq = 1.0 / (10000.0 ** (np.arange(0, HD, 2, dtype=np.float64) / HD))
    t = np.arange(S, dtype=np.float64)
    freqs = t[:, None] * inv_freq[None, :]            # [S, 64]
    cosT = np.cos(freqs).T.astype(np.float32)         # [64, S]
    sinT = np.sin(freqs).T.astype(np.float32)
    cosS = np.ascontiguousarray(np.vstack([cosT, cosT])).astype(bf16)
    sinS = np.ascontiguousarray(np.vstack([-sinT, sinT])).astype(bf16)

    # triangular causal mask for the diagonal 128x128 block
    pidx = np.arange(P)[:, None]
    qidx = np.arange(P)[None, :]
    m = np.ascontiguousarray((qidx >= pidx).astype(bf16))

    ones = np.ones((P, 1), dtype=bf16)

    in_maps = []
    for c in range(NCORES):
        fs = slice(c * FPC, (c + 1) * FPC)
        in_maps.append({
            "xT": xT,
            "wqT": np.ascontiguousarray(wq_p[fs].T).astype(bf16),   # [D, 256]
            "wkT": np.ascontiguousarray(wk_p[fs].T).astype(bf16),
            "wvT": np.ascontiguousarray(wv[fs].T).astype(bf16),
            "woT": np.ascontiguousarray(wo[:, fs].T).astype(bf16),  # [256, D]
            "cosS": cosS,
            "sinS": sinS,
            "masks": m,
            "onesd": ones,
            "identd": np.eye(P, dtype=bf16),
        })
    return in_maps


def _run(inputs, trace=False):
    from concourse.bass_utils import run_bass_kernel_spmd

    if "nc" not in _CACHE:
        _CACHE["nc"] = _build_nc()
    nc = _CACHE["nc"]

    in_maps = _host_prep(
        inputs["x"], inputs["wq"], inputs["wk"], inputs["wv"], inputs["wo"]
    )
    res = run_bass_kernel_spmd(nc, in_maps, list(range(NCORES)), trace=trace)
    acc = None
    for c in range(NCORES):
        part = np.asarray(res.results[c]["outp"], dtype=np.float32)
        acc = part.copy() if acc is None else acc + part
    out = acc.reshape(B, S, D).astype(np.float32)
    return out, res


def kernel(**inputs) -> np.ndarray:
    out, _ = _run(inputs, trace=False)
    return out
